# revision 27
# baseline (speedup 1.0000x reference)
"""RIENet loss kernel (keypoint/KNN MSE + global-align Huber-min loss) on 8 trn2 cores.

Sharding: core ci -> (b = ci // 4, n-chunk j = ci % 4).  Each core holds the full
tgt[b] (M=8192 points) and a 2048-column chunk of src_transformed[b] (N axis).
  loss_1 (min over M per src point): complete locally per core.
  loss_2 (min over N per tgt point): per-core partial min over its chunk;
          host min-reduces the 4 chunks per batch element.

v4 — PE row-tiling (4x 32x128 tiles) + three-engine pipeline:
  Host builds grouped lhsT (tAg [128, 2048]) and 4x-replicated rhs
  (sAr [128, 2048]) bf16 factor matrices so one K=24 matmul per PE
  row-tile yields Q[m, n] = ||t_m - s_n||^2 in PSUM f32.  Four PE tiles
  (partition bases 0/32/64/96) process four m-tiles concurrently.
  Per group g (4 m-tiles), per nj (512-col slice of the chunk):
    PE:   4 concurrent matmuls -> quad [128, 4x512]    (4 psum banks)
    ACT:  copy quad f32 -> q16 slice fp16              (~1.8 us)  <- bottleneck
  Per group: DVE 4x TT-min into acc (2x mode) + one 4D reduce_min
  (4x mode) -> rowbuf[:, 4g:4g+4].
  acc is partition-min-reduced via PE transposes at the end.  Tiny
  keypoint/KNN MSE losses run on-device on every core.
"""

import os
import numpy as np
import ml_dtypes


def _ensure_path():
    try:
        import concourse  # noqa: F401
    except ImportError:
        import sys
        for p in ("/opt/trn_rl_repo", "/root/.axon_site/_ro/trn_rl_repo"):
            if os.path.isdir(p) and p not in sys.path:
                sys.path.insert(0, p)


_ensure_path()

import concourse.bass as bass  # noqa: E402
import concourse.bacc as bacc  # noqa: E402
import concourse.tile as tile  # noqa: E402
import concourse.mybir as mybir  # noqa: E402
from concourse.bass_utils import run_bass_kernel_spmd  # noqa: E402

F32 = mybir.dt.float32
F16 = mybir.dt.float16
BF16 = mybir.dt.bfloat16
AL = mybir.AluOpType
BF = ml_dtypes.bfloat16

MARGIN = 0.1
B, KP, KNN, N, M = 2, 256, 32, 8192, 8192
NCORES = 8
NSHARDS = NCORES // B          # 4 n-chunks per batch element
CHUNK = N // NSHARDS           # 2048
NJ = CHUNK // 512              # 4 psum banks per quad
MI = M // 128                  # 64 m-tiles
NG = MI // 4                   # 16 groups of 4 m-tiles
K24 = 24
ACC_INIT = 60000.0             # > max possible distance^2, fp16-representable

_CACHE = {}


def _build():
    nc = bacc.Bacc("TRN2", target_bir_lowering=False, debug=False,
                   num_devices=NCORES)

    tAg_d = nc.dram_tensor("tAg", [128, M // 4], BF16, kind="ExternalInput")
    sAr_d = nc.dram_tensor("sAr", [128, CHUNK], BF16, kind="ExternalInput")
    ident = nc.dram_tensor("ident", [128, 128], F16, kind="ExternalInput")
    kp_lhsT = nc.dram_tensor("kp_lhsT", [4, 2 * 3], F32, kind="ExternalInput")
    kp_rhs = nc.dram_tensor("kp_rhs", [4, 2 * KP], F32, kind="ExternalInput")
    tgt_kp = nc.dram_tensor("tgt_kp", [3, 2 * KP], F32, kind="ExternalInput")
    knn_src = nc.dram_tensor("knn_src", [128, 2 * 192], F32, kind="ExternalInput")
    knn_tgt = nc.dram_tensor("knn_tgt", [128, 2 * 192], F32, kind="ExternalInput")

    colmin_o = nc.dram_tensor("colmin", [128, CHUNK // 128], F32, kind="ExternalOutput")
    rowmin_o = nc.dram_tensor("rowmin", [128, MI], F16, kind="ExternalOutput")
    misc_o = nc.dram_tensor("misc", [128, 4], F32, kind="ExternalOutput")

    with tile.TileContext(nc) as tc:
        with (
            tc.tile_pool(name="const", bufs=1) as const,
            tc.tile_pool(name="sc", bufs=2) as sc,
            tc.tile_pool(name="kd", bufs=2) as kd,
        ):
            tAg = const.tile([128, M // 4], BF16)
            sAr = const.tile([128, CHUNK], BF16)
            acc = const.tile([128, CHUNK], F16)
            rowbuf = const.tile([128, MI], F16)
            id_sb = const.tile([128, 128], F16)
            colmin_sb = const.tile([128, CHUNK // 128], F32)
            misc_sb = const.tile([128, 4], F32)

            nc.sync.dma_start(out=tAg[:], in_=tAg_d[:])
            nc.scalar.dma_start(out=sAr[:], in_=sAr_d[:])
            nc.sync.dma_start(out=id_sb[:], in_=ident[:])
            kp_l = const.tile([4, 2 * 3], F32)
            kp_r = const.tile([4, 2 * KP], F32)
            kp_t = const.tile([3, 2 * KP], F32)
            ks = const.tile([128, 2 * 192], F32)
            kt = const.tile([128, 2 * 192], F32)
            nc.sync.dma_start(out=kp_l[:], in_=kp_lhsT[:])
            nc.sync.dma_start(out=kp_r[:], in_=kp_rhs[:])
            nc.sync.dma_start(out=kp_t[:], in_=tgt_kp[:])
            nc.sync.dma_start(out=ks[:], in_=knn_src[:])
            nc.sync.dma_start(out=kt[:], in_=knn_tgt[:])
            nc.gpsimd.memset(acc[:], ACC_INIT)
            nc.gpsimd.memset(misc_sb[:], 0.0)

            # ---- main loop: 16 groups x 4 m-tiles via PE row tiling ----
            # quad r = m-tile 4g+r over the full chunk; PE tile r keeps the
            # same weights for 4 consecutive matmuls.
            with tc.tile_pool(name="psum_main", bufs=2, space="PSUM") as pm:
                for g in range(NG):
                    q16 = sc.tile([128, 4 * CHUNK], F16, tag="q16")
                    for r in range(4):
                        quad = pm.tile([128, CHUNK], F32, tag="quad")
                        for nj in range(NJ):
                            nc.tensor.matmul(
                                quad[:, nj * 512:(nj + 1) * 512],
                                lhsT=tAg[32 * r:32 * r + K24,
                                         g * 128:(g + 1) * 128],
                                rhs=sAr[32 * r:32 * r + K24,
                                        nj * 512:(nj + 1) * 512],
                                start=True, stop=True,
                                tile_position=(32 * r, 0),
                            )
                        nc.scalar.copy(
                            out=q16[:, r * CHUNK:(r + 1) * CHUNK],
                            in_=quad[:])
                    # q16[p, r*2048 + n] = Q[(4g+r)*128+p, n]
                    # colmin: pairwise-tree min over the 4 m-tiles, then one
                    # chained min into acc (all TT, 2x mode)
                    cu = sc.tile([128, 2 * CHUNK], F16, tag="cu")
                    nc.vector.tensor_tensor(cu[:], q16[:, :2 * CHUNK],
                                            q16[:, 2 * CHUNK:], AL.min)
                    cw = sc.tile([128, CHUNK], F16, tag="cw")
                    nc.vector.tensor_tensor(cw[:], cu[:, :CHUNK],
                                            cu[:, CHUNK:], AL.min)
                    nc.vector.tensor_tensor(acc[:], cw[:], acc[:], AL.min)
                    # rowmin of the 4 m-tiles: TT-min tree over n (2x mode)
                    # with per-r segmentation, then a small 1x tail reduce.
                    qvv = q16.rearrange("p (r h c) -> p r h c", r=4, h=2,
                                        c=CHUNK // 2)
                    ta = sc.tile([128, 4, CHUNK // 2], F16, tag="ta")
                    nc.vector.tensor_tensor(ta[:], qvv[:, :, 0], qvv[:, :, 1],
                                            AL.min)
                    tav = ta.rearrange("p r (h c) -> p r h c", h=2,
                                       c=CHUNK // 4)
                    tb = sc.tile([128, 4, CHUNK // 4], F16, tag="tb")
                    nc.vector.tensor_tensor(tb[:], tav[:, :, 0], tav[:, :, 1],
                                            AL.min)
                    tbv = tb.rearrange("p r (h c) -> p r h c", h=2,
                                       c=CHUNK // 8)
                    td = sc.tile([128, 4, CHUNK // 8], F16, tag="td")
                    nc.vector.tensor_tensor(td[:], tbv[:, :, 0], tbv[:, :, 1],
                                            AL.min)
                    tdv = td.rearrange("p r (h c) -> p r h c", h=2,
                                       c=CHUNK // 16)
                    te = sc.tile([128, 4, CHUNK // 16], F16, tag="te")
                    nc.vector.tensor_tensor(te[:], tdv[:, :, 0], tdv[:, :, 1],
                                            AL.min)
                    nc.vector.tensor_reduce(
                        out=rowbuf[:, g * 4:(g + 1) * 4], in_=te[:],
                        axis=mybir.AxisListType.X, op=AL.min)

            with tc.tile_pool(name="psum_fin", bufs=1, space="PSUM") as pf:
                # partition-axis min of acc: 16 PE transposes into one fp16
                # psum tile (2 banks), then a single segmented reduce
                tp = pf.tile([128, CHUNK // 128, 128], F16, tag="tp")
                for blk in range(CHUNK // 128):
                    nc.tensor.transpose(tp[:, blk],
                                        acc[:, blk * 128:(blk + 1) * 128],
                                        id_sb[:])
                # tiny keypoint / knn losses (fill the transpose wait)
                for b in range(B):
                    pt2 = pf.tile([3, KP], F32, tag="kp")
                    nc.tensor.matmul(
                        pt2[:], lhsT=kp_l[:, b * 3:(b + 1) * 3],
                        rhs=kp_r[:, b * KP:(b + 1) * KP],
                        start=True, stop=True)
                    diff = kd.tile([3, KP], F32, tag="kdiff")
                    nc.vector.tensor_sub(diff[:], pt2[:],
                                         kp_t[:, b * KP:(b + 1) * KP])
                    nc.vector.tensor_mul(diff[:], diff[:], diff[:])
                    nc.vector.tensor_reduce(
                        out=misc_sb[0:3, b:b + 1], in_=diff[:],
                        axis=mybir.AxisListType.X, op=AL.add)
                    diff2 = kd.tile([128, 192], F32, tag="ndiff")
                    nc.vector.tensor_sub(diff2[:], ks[:, b * 192:(b + 1) * 192],
                                         kt[:, b * 192:(b + 1) * 192])
                    nc.vector.tensor_mul(diff2[:], diff2[:], diff2[:])
                    nc.vector.tensor_reduce(
                        out=misc_sb[:, 2 + b:3 + b], in_=diff2[:],
                        axis=mybir.AxisListType.X, op=AL.add)
                nc.vector.tensor_reduce(
                    out=colmin_sb[:], in_=tp[:],
                    axis=mybir.AxisListType.X, op=AL.min)

            nc.sync.dma_start(out=misc_o[:], in_=misc_sb[:])
            nc.sync.dma_start(out=colmin_o[:], in_=colmin_sb[:])
            nc.sync.dma_start(out=rowmin_o[:], in_=rowbuf[:])

    nc.compile()
    return nc


def _get_nc():
    if "nc" not in _CACHE:
        _CACHE["nc"] = _build()
    return _CACHE["nc"]


def _split3(x):
    h = x.astype(BF).astype(np.float32)
    r = x - h
    m = r.astype(BF).astype(np.float32)
    l = (r - m).astype(BF).astype(np.float32)
    return h, m, l


def _build_ops(t, s):
    # t (3, M), s (3, CHUNK) f32 -> tA [24, M], sA [24, CHUNK] f32 with
    # sum_k tA[k, m] * sA[k, n] ~= ||t_m - s_n||^2
    tm2 = -2.0 * t
    nt = (t * t).sum(0)
    ns = (s * s).sum(0)
    th, tm, tl = _split3(tm2)
    sh, sm, sl = _split3(s)
    nth, ntm, ntl = _split3(nt)
    nsh, nsm, nsl = _split3(ns)
    tA = np.zeros((K24, t.shape[1]), np.float32)
    sA = np.zeros((K24, s.shape[1]), np.float32)
    pairs = [(th, sh), (th, sm), (tm, sh), (tm, sm), (th, sl), (tl, sh)]
    for pi, (ta, sa) in enumerate(pairs):
        for d in range(3):
            tA[pi * 3 + d] = ta[d]
            sA[pi * 3 + d] = sa[d]
    tA[18:21] = 1.0
    sA[18], sA[19], sA[20] = nsh, nsm, nsl
    tA[21], tA[22], tA[23] = nth, ntm, ntl
    sA[21:24] = 1.0

    # group lhsT for 4x PE row tiling: tile r handles m-tiles mi = 4g + r
    tAg = np.zeros((128, M // 4), np.float32)
    tAv = tA.reshape(K24, NG, 4, 128)
    for r in range(4):
        tAg[32 * r:32 * r + K24, :] = tAv[:, :, r, :].reshape(K24, NG * 128)
    sAr = np.zeros((128, CHUNK), np.float32)
    for r in range(4):
        sAr[32 * r:32 * r + K24, :] = sA
    return (np.ascontiguousarray(tAg.astype(BF)),
            np.ascontiguousarray(sAr.astype(BF)))


def _prepare_in_maps(src_keypoints, tgt_keypoints, rotation_ab, translation_ab,
                     src_keypoints_knn, tgt_keypoints_knn, src_transformed, tgt):
    f = np.float32
    st = np.ascontiguousarray(np.asarray(src_transformed, dtype=f))
    tg = np.ascontiguousarray(np.asarray(tgt, dtype=f))
    skp = np.asarray(src_keypoints, dtype=f)
    tkp = np.asarray(tgt_keypoints, dtype=f)
    rot = np.asarray(rotation_ab, dtype=f)
    tra = np.asarray(translation_ab, dtype=f)
    sknn = np.asarray(src_keypoints_knn, dtype=f)
    tknn = np.asarray(tgt_keypoints_knn, dtype=f)

    ident = np.eye(128, dtype=np.float16)
    kp_lhsT = np.zeros((4, 2 * 3), dtype=f)
    kp_rhs = np.zeros((4, 2 * KP), dtype=f)
    tgt_kp = np.zeros((3, 2 * KP), dtype=f)
    knn_src = np.zeros((128, 2 * 192), dtype=f)
    knn_tgt = np.zeros((128, 2 * 192), dtype=f)
    for b in range(B):
        kp_lhsT[0:3, b * 3:(b + 1) * 3] = rot[b].T
        kp_lhsT[3, b * 3:(b + 1) * 3] = tra[b]
        kp_rhs[0:3, b * KP:(b + 1) * KP] = skp[b]
        kp_rhs[3, b * KP:(b + 1) * KP] = 1.0
        tgt_kp[:, b * KP:(b + 1) * KP] = tkp[b]
        knn_src[:, b * 192:(b + 1) * 192] = sknn[b].reshape(128, 192)
        knn_tgt[:, b * 192:(b + 1) * 192] = tknn[b].reshape(128, 192)

    shared = {
        "ident": ident, "kp_lhsT": kp_lhsT, "kp_rhs": kp_rhs,
        "tgt_kp": tgt_kp, "knn_src": knn_src, "knn_tgt": knn_tgt,
    }
    in_maps = []
    for ci in range(NCORES):
        b, j = divmod(ci, NSHARDS)
        m = dict(shared)
        tAg, sAr = _build_ops(tg[b], st[b][:, j * CHUNK:(j + 1) * CHUNK])
        m["tAg"] = tAg
        m["sAr"] = sAr
        in_maps.append(m)
    return in_maps


def _huber(x, c):
    return np.where(x < c, 0.5 * x * x, c * x - 0.5 * c * c)


def _postprocess(results):
    c = np.float64(MARGIN)
    loss1 = np.float64(0.0)
    loss2 = np.float64(0.0)
    for b in range(B):
        rowmins = []
        for j in range(NSHARDS):
            r = results[b * NSHARDS + j]
            colmin = np.asarray(r["colmin"], dtype=np.float64).T.ravel()
            loss1 += _huber(colmin, c).sum()
            rowmins.append(np.asarray(r["rowmin"], dtype=np.float64).T.ravel())
        rm = np.minimum.reduce(rowmins)
        loss2 += _huber(rm, c).sum()
    gal = loss1 + loss2

    misc = np.asarray(results[0]["misc"], dtype=np.float64)
    kp_loss = (misc[0:3, 0].sum() + misc[0:3, 1].sum()) / B
    knn_loss = (misc[:, 2].sum() + misc[:, 3].sum()) / (B * KNN)
    ncl = knn_loss + kp_loss
    return np.float32(ncl), np.float32(gal)


def run_device(in_maps, **kw):
    nc = _get_nc()
    return run_bass_kernel_spmd(nc, in_maps, list(range(NCORES)), **kw)


def kernel(src_keypoints, tgt_keypoints, rotation_ab, translation_ab,
           src_keypoints_knn, tgt_keypoints_knn, k, src_transformed, tgt,
           **_unused):
    in_maps = _prepare_in_maps(src_keypoints, tgt_keypoints, rotation_ab,
                               translation_ab, src_keypoints_knn,
                               tgt_keypoints_knn, src_transformed, tgt)
    res = run_device(in_maps)
    return _postprocess(res.results)


# revision 28
# speedup vs baseline: 1.1951x; 1.1951x over previous
"""RIENet loss kernel (keypoint/KNN MSE + global-align Huber-min loss) on 8 trn2 cores.

Sharding: core ci -> (b = ci // 4, n-chunk j = ci % 4).  Each core holds the full
tgt[b] (M=8192 points) and a 2048-column chunk of src_transformed[b] (N axis).
  loss_1 (min over M per src point): complete locally per core.
  loss_2 (min over N per tgt point): per-core partial min over its chunk;
          host min-reduces the 4 chunks per batch element.

v4 — PE row-tiling (4x 32x128 tiles) + three-engine pipeline:
  Host builds grouped lhsT (tAg [128, 2048]) and 4x-replicated rhs
  (sAr [128, 2048]) bf16 factor matrices so one K=24 matmul per PE
  row-tile yields Q[m, n] = ||t_m - s_n||^2 in PSUM f32.  Four PE tiles
  (partition bases 0/32/64/96) process four m-tiles concurrently.
  Per group g (4 m-tiles), per nj (512-col slice of the chunk):
    PE:   4 concurrent matmuls -> quad [128, 4x512]    (4 psum banks)
    ACT:  copy quad f32 -> q16 slice fp16              (~1.8 us)  <- bottleneck
  Per group: DVE 4x TT-min into acc (2x mode) + one 4D reduce_min
  (4x mode) -> rowbuf[:, 4g:4g+4].
  acc is partition-min-reduced via PE transposes at the end.  Tiny
  keypoint/KNN MSE losses run on-device on every core.
"""

import os
import numpy as np
import ml_dtypes


def _ensure_path():
    try:
        import concourse  # noqa: F401
    except ImportError:
        import sys
        for p in ("/opt/trn_rl_repo", "/root/.axon_site/_ro/trn_rl_repo"):
            if os.path.isdir(p) and p not in sys.path:
                sys.path.insert(0, p)


_ensure_path()

import concourse.bass as bass  # noqa: E402
import concourse.bacc as bacc  # noqa: E402
import concourse.tile as tile  # noqa: E402
import concourse.mybir as mybir  # noqa: E402
from concourse.bass_utils import run_bass_kernel_spmd  # noqa: E402

F32 = mybir.dt.float32
F16 = mybir.dt.float16
BF16 = mybir.dt.bfloat16
AL = mybir.AluOpType
BF = ml_dtypes.bfloat16

MARGIN = 0.1
B, KP, KNN, N, M = 2, 256, 32, 8192, 8192
NCORES = 8
NSHARDS = NCORES // B          # 4 n-chunks per batch element
CHUNK = N // NSHARDS           # 2048
NJ = CHUNK // 512              # 4 psum banks per quad
MI = M // 128                  # 64 m-tiles
NG = MI // 4                   # 16 groups of 4 m-tiles
K24 = 24
ACC_INIT = 60000.0             # > max possible distance^2, fp16-representable

_CACHE = {}


def _build():
    nc = bacc.Bacc("TRN2", target_bir_lowering=False, debug=False,
                   num_devices=NCORES)

    tAg_d = nc.dram_tensor("tAg", [128, M // 4], BF16, kind="ExternalInput")
    sAr_d = nc.dram_tensor("sAr", [128, CHUNK], BF16, kind="ExternalInput")
    ident = nc.dram_tensor("ident", [128, 128], F16, kind="ExternalInput")
    kp_lhsT = nc.dram_tensor("kp_lhsT", [4, 2 * 3], F32, kind="ExternalInput")
    kp_rhs = nc.dram_tensor("kp_rhs", [4, 2 * KP], F32, kind="ExternalInput")
    tgt_kp = nc.dram_tensor("tgt_kp", [3, 2 * KP], F32, kind="ExternalInput")
    knn_src = nc.dram_tensor("knn_src", [128, 2 * 192], F32, kind="ExternalInput")
    knn_tgt = nc.dram_tensor("knn_tgt", [128, 2 * 192], F32, kind="ExternalInput")

    colmin_o = nc.dram_tensor("colmin", [128, CHUNK // 128], F32, kind="ExternalOutput")
    rowmin_o = nc.dram_tensor("rowmin", [128, MI], F16, kind="ExternalOutput")
    misc_o = nc.dram_tensor("misc", [128, 4], F32, kind="ExternalOutput")

    with tile.TileContext(nc) as tc:
        with (
            tc.tile_pool(name="const", bufs=1) as const,
            tc.tile_pool(name="sc", bufs=2) as sc,
            tc.tile_pool(name="kd", bufs=2) as kd,
        ):
            tAg = const.tile([128, M // 4], BF16)
            sAr = const.tile([128, CHUNK], BF16)
            acc = const.tile([128, CHUNK], F16)
            rowbuf = const.tile([128, MI], F16)
            id_sb = const.tile([128, 128], F16)
            colmin_sb = const.tile([128, CHUNK // 128], F32)
            misc_sb = const.tile([128, 4], F32)

            nc.sync.dma_start(out=tAg[:], in_=tAg_d[:])
            nc.sync.dma_start(out=sAr[:], in_=sAr_d[:])
            nc.sync.dma_start(out=id_sb[:], in_=ident[:])
            nc.gpsimd.memset(acc[:], ACC_INIT)
            nc.gpsimd.memset(misc_sb[:], 0.0)

            # ---- tiny keypoint / knn losses first (overlap input DMAs) ----
            with tc.tile_pool(name="psum_kp", bufs=2, space="PSUM") as pk:
                kp_l = const.tile([4, 2 * 3], F32)
                kp_r = const.tile([4, 2 * KP], F32)
                kp_t = const.tile([3, 2 * KP], F32)
                ks = const.tile([128, 2 * 192], F32)
                kt = const.tile([128, 2 * 192], F32)
                nc.scalar.dma_start(out=kp_l[:], in_=kp_lhsT[:])
                nc.scalar.dma_start(out=kp_r[:], in_=kp_rhs[:])
                nc.scalar.dma_start(out=kp_t[:], in_=tgt_kp[:])
                nc.scalar.dma_start(out=ks[:], in_=knn_src[:])
                nc.scalar.dma_start(out=kt[:], in_=knn_tgt[:])
                for b in range(B):
                    pt2 = pk.tile([3, KP], F32, tag="kp")
                    nc.tensor.matmul(
                        pt2[:], lhsT=kp_l[:, b * 3:(b + 1) * 3],
                        rhs=kp_r[:, b * KP:(b + 1) * KP],
                        start=True, stop=True)
                    diff = kd.tile([3, KP], F32, tag="kdiff")
                    nc.vector.tensor_sub(diff[:], pt2[:],
                                         kp_t[:, b * KP:(b + 1) * KP])
                    nc.vector.tensor_mul(diff[:], diff[:], diff[:])
                    nc.vector.tensor_reduce(
                        out=misc_sb[0:3, b:b + 1], in_=diff[:],
                        axis=mybir.AxisListType.X, op=AL.add)
                    diff2 = kd.tile([128, 192], F32, tag="ndiff")
                    nc.vector.tensor_sub(diff2[:], ks[:, b * 192:(b + 1) * 192],
                                         kt[:, b * 192:(b + 1) * 192])
                    nc.vector.tensor_mul(diff2[:], diff2[:], diff2[:])
                    nc.vector.tensor_reduce(
                        out=misc_sb[:, 2 + b:3 + b], in_=diff2[:],
                        axis=mybir.AxisListType.X, op=AL.add)
            nc.sync.dma_start(out=misc_o[:], in_=misc_sb[:])

            # ---- main loop: 16 groups x 4 m-tiles via PE row tiling ----
            # quad r = m-tile 4g+r over the full chunk; PE tile r keeps the
            # same weights for 4 consecutive matmuls.
            with tc.tile_pool(name="psum_main", bufs=2, space="PSUM") as pm:
                for g in range(NG):
                    q16 = sc.tile([128, 4 * CHUNK], F16, tag="q16")
                    for r in range(4):
                        quad = pm.tile([128, CHUNK], F32, tag="quad")
                        for nj in range(NJ):
                            nc.tensor.matmul(
                                quad[:, nj * 512:(nj + 1) * 512],
                                lhsT=tAg[32 * r:32 * r + K24,
                                         g * 128:(g + 1) * 128],
                                rhs=sAr[32 * r:32 * r + K24,
                                        nj * 512:(nj + 1) * 512],
                                start=True, stop=True,
                                tile_position=(32 * r, 0),
                            )
                        nc.scalar.copy(
                            out=q16[:, r * CHUNK:(r + 1) * CHUNK],
                            in_=quad[:])
                    # q16[p, r*2048 + n] = Q[(4g+r)*128+p, n]
                    # colmin: pairwise-tree min over the 4 m-tiles, then one
                    # chained min into acc (all TT, 2x mode)
                    cu = sc.tile([128, 2 * CHUNK], F16, tag="cu")
                    nc.vector.tensor_tensor(cu[:], q16[:, :2 * CHUNK],
                                            q16[:, 2 * CHUNK:], AL.min)
                    cw = sc.tile([128, CHUNK], F16, tag="cw")
                    nc.vector.tensor_tensor(cw[:], cu[:, :CHUNK],
                                            cu[:, CHUNK:], AL.min)
                    nc.vector.tensor_tensor(acc[:], cw[:], acc[:], AL.min)
                    # rowmin of the 4 m-tiles: TT-min tree over n (2x mode)
                    # with per-r segmentation, then a small 1x tail reduce.
                    qvv = q16.rearrange("p (r h c) -> p r h c", r=4, h=2,
                                        c=CHUNK // 2)
                    ta = sc.tile([128, 4, CHUNK // 2], F16, tag="ta")
                    nc.vector.tensor_tensor(ta[:], qvv[:, :, 0], qvv[:, :, 1],
                                            AL.min)
                    tav = ta.rearrange("p r (h c) -> p r h c", h=2,
                                       c=CHUNK // 4)
                    tb = sc.tile([128, 4, CHUNK // 4], F16, tag="tb")
                    nc.vector.tensor_tensor(tb[:], tav[:, :, 0], tav[:, :, 1],
                                            AL.min)
                    tbv = tb.rearrange("p r (h c) -> p r h c", h=2,
                                       c=CHUNK // 8)
                    td = sc.tile([128, 4, CHUNK // 8], F16, tag="td")
                    nc.vector.tensor_tensor(td[:], tbv[:, :, 0], tbv[:, :, 1],
                                            AL.min)
                    tdv = td.rearrange("p r (h c) -> p r h c", h=2,
                                       c=CHUNK // 16)
                    te = sc.tile([128, 4, CHUNK // 16], F16, tag="te")
                    nc.vector.tensor_tensor(te[:], tdv[:, :, 0], tdv[:, :, 1],
                                            AL.min)
                    nc.vector.tensor_reduce(
                        out=rowbuf[:, g * 4:(g + 1) * 4], in_=te[:],
                        axis=mybir.AxisListType.X, op=AL.min)

            with tc.tile_pool(name="psum_fin", bufs=1, space="PSUM") as pf:
                # partition-axis min of acc: 16 PE transposes into one fp16
                # psum tile (2 banks), then a single segmented reduce
                tp = pf.tile([128, CHUNK // 128, 128], F16, tag="tp")
                for blk in range(CHUNK // 128):
                    nc.tensor.transpose(tp[:, blk],
                                        acc[:, blk * 128:(blk + 1) * 128],
                                        id_sb[:])
                nc.vector.tensor_reduce(
                    out=colmin_sb[:], in_=tp[:],
                    axis=mybir.AxisListType.X, op=AL.min)

            nc.sync.dma_start(out=colmin_o[:], in_=colmin_sb[:])
            nc.sync.dma_start(out=rowmin_o[:], in_=rowbuf[:])

    nc.compile()
    return nc


def _get_nc():
    if "nc" not in _CACHE:
        _CACHE["nc"] = _build()
    return _CACHE["nc"]


def _split3(x):
    h = x.astype(BF).astype(np.float32)
    r = x - h
    m = r.astype(BF).astype(np.float32)
    l = (r - m).astype(BF).astype(np.float32)
    return h, m, l


def _build_ops(t, s):
    # t (3, M), s (3, CHUNK) f32 -> tA [24, M], sA [24, CHUNK] f32 with
    # sum_k tA[k, m] * sA[k, n] ~= ||t_m - s_n||^2
    tm2 = -2.0 * t
    nt = (t * t).sum(0)
    ns = (s * s).sum(0)
    th, tm, tl = _split3(tm2)
    sh, sm, sl = _split3(s)
    nth, ntm, ntl = _split3(nt)
    nsh, nsm, nsl = _split3(ns)
    tA = np.zeros((K24, t.shape[1]), np.float32)
    sA = np.zeros((K24, s.shape[1]), np.float32)
    pairs = [(th, sh), (th, sm), (tm, sh), (tm, sm), (th, sl), (tl, sh)]
    for pi, (ta, sa) in enumerate(pairs):
        for d in range(3):
            tA[pi * 3 + d] = ta[d]
            sA[pi * 3 + d] = sa[d]
    tA[18:21] = 1.0
    sA[18], sA[19], sA[20] = nsh, nsm, nsl
    tA[21], tA[22], tA[23] = nth, ntm, ntl
    sA[21:24] = 1.0

    # group lhsT for 4x PE row tiling: tile r handles m-tiles mi = 4g + r
    tAg = np.zeros((128, M // 4), np.float32)
    tAv = tA.reshape(K24, NG, 4, 128)
    for r in range(4):
        tAg[32 * r:32 * r + K24, :] = tAv[:, :, r, :].reshape(K24, NG * 128)
    sAr = np.zeros((128, CHUNK), np.float32)
    for r in range(4):
        sAr[32 * r:32 * r + K24, :] = sA
    return (np.ascontiguousarray(tAg.astype(BF)),
            np.ascontiguousarray(sAr.astype(BF)))


def _prepare_in_maps(src_keypoints, tgt_keypoints, rotation_ab, translation_ab,
                     src_keypoints_knn, tgt_keypoints_knn, src_transformed, tgt):
    f = np.float32
    st = np.ascontiguousarray(np.asarray(src_transformed, dtype=f))
    tg = np.ascontiguousarray(np.asarray(tgt, dtype=f))
    skp = np.asarray(src_keypoints, dtype=f)
    tkp = np.asarray(tgt_keypoints, dtype=f)
    rot = np.asarray(rotation_ab, dtype=f)
    tra = np.asarray(translation_ab, dtype=f)
    sknn = np.asarray(src_keypoints_knn, dtype=f)
    tknn = np.asarray(tgt_keypoints_knn, dtype=f)

    ident = np.eye(128, dtype=np.float16)
    kp_lhsT = np.zeros((4, 2 * 3), dtype=f)
    kp_rhs = np.zeros((4, 2 * KP), dtype=f)
    tgt_kp = np.zeros((3, 2 * KP), dtype=f)
    knn_src = np.zeros((128, 2 * 192), dtype=f)
    knn_tgt = np.zeros((128, 2 * 192), dtype=f)
    for b in range(B):
        kp_lhsT[0:3, b * 3:(b + 1) * 3] = rot[b].T
        kp_lhsT[3, b * 3:(b + 1) * 3] = tra[b]
        kp_rhs[0:3, b * KP:(b + 1) * KP] = skp[b]
        kp_rhs[3, b * KP:(b + 1) * KP] = 1.0
        tgt_kp[:, b * KP:(b + 1) * KP] = tkp[b]
        knn_src[:, b * 192:(b + 1) * 192] = sknn[b].reshape(128, 192)
        knn_tgt[:, b * 192:(b + 1) * 192] = tknn[b].reshape(128, 192)

    shared = {
        "ident": ident, "kp_lhsT": kp_lhsT, "kp_rhs": kp_rhs,
        "tgt_kp": tgt_kp, "knn_src": knn_src, "knn_tgt": knn_tgt,
    }
    in_maps = []
    for ci in range(NCORES):
        b, j = divmod(ci, NSHARDS)
        m = dict(shared)
        tAg, sAr = _build_ops(tg[b], st[b][:, j * CHUNK:(j + 1) * CHUNK])
        m["tAg"] = tAg
        m["sAr"] = sAr
        in_maps.append(m)
    return in_maps


def _huber(x, c):
    return np.where(x < c, 0.5 * x * x, c * x - 0.5 * c * c)


def _postprocess(results):
    c = np.float64(MARGIN)
    loss1 = np.float64(0.0)
    loss2 = np.float64(0.0)
    for b in range(B):
        rowmins = []
        for j in range(NSHARDS):
            r = results[b * NSHARDS + j]
            colmin = np.asarray(r["colmin"], dtype=np.float64).T.ravel()
            loss1 += _huber(colmin, c).sum()
            rowmins.append(np.asarray(r["rowmin"], dtype=np.float64).T.ravel())
        rm = np.minimum.reduce(rowmins)
        loss2 += _huber(rm, c).sum()
    gal = loss1 + loss2

    misc = np.asarray(results[0]["misc"], dtype=np.float64)
    kp_loss = (misc[0:3, 0].sum() + misc[0:3, 1].sum()) / B
    knn_loss = (misc[:, 2].sum() + misc[:, 3].sum()) / (B * KNN)
    ncl = knn_loss + kp_loss
    return np.float32(ncl), np.float32(gal)


def run_device(in_maps, **kw):
    nc = _get_nc()
    return run_bass_kernel_spmd(nc, in_maps, list(range(NCORES)), **kw)


def kernel(src_keypoints, tgt_keypoints, rotation_ab, translation_ab,
           src_keypoints_knn, tgt_keypoints_knn, k, src_transformed, tgt,
           **_unused):
    in_maps = _prepare_in_maps(src_keypoints, tgt_keypoints, rotation_ab,
                               translation_ab, src_keypoints_knn,
                               tgt_keypoints_knn, src_transformed, tgt)
    res = run_device(in_maps)
    return _postprocess(res.results)
